# revision 9
# baseline (speedup 1.0000x reference)
"""AttentiveProtoFusion kernel for 8 TRN2 NeuronCores.

Math (equivalent to reference, ~14x fewer FLOPs):
    q' = sent @ (Wq @ Wk^T) + bq @ Wk^T      [n, 768]   (folded host-side)
    scores[n,p] = sum_c proto[n,p,c] * q'[n,c]
    w = softmax(scores, axis=p);  ctx[n,c] = sum_p w[n,p] * proto[n,p,c]

Sharding: data-parallel over the 2048 tokens (B*S), 256/core, 2 blocks of
128 tokens (tokens on partitions). proto/sent/W staged host-side in fp16
(rel err ~2.8e-3 vs the 2e-2 gate; halves DMA to 12 MiB/core).

Softmax uses the fixed exponent frame proven in the fp32 baseline:
Mhat = max(chunk0) + 60, scores clamped at Mhat + 80, so e = exp(s - Mhat)
spans up to e^80 = 5.5e34 - safely inside bf16 range. The pooling weights
e are materialised as bf16 DIAGONAL matrices and the whole MAC
U += e_p * proto_p runs on the TensorEngine as
matmul(lhsT=diag(e_p) bf16, rhs=proto_p fp16) accumulating in PSUM fp32
(mixed 16-bit dtypes verified exact on HW; ldweights pipelines behind the
previous matmul, 163 ns/384-col matmul at full clock).

Engine plan (measured costs):
  DVE   : score passes via fused scalar_tensor_tensor+accum (~1.1 us);
          some diag builds diag=TS(eye*e_p) (~345 ns); softmax sm/frames.
  GPSIMD: score products for ~13 of 32 protos per block (TT ~1.65 us).
  ACT   : accumulates GPSIMD products (~1.3 us); per-chunk exp for Z;
          other diag builds via dg=Exp(eyeNEG + sm_p) bf16 (~385 ns,
          eyeNEG has -60000 off-diagonal so exp -> 0); q' copies; final
          ctx = U * (1/Z) PSUM->SBUF copies.
  PE    : q' projection + the 128 MAC matmuls/block.
  DMA   : proto fp16 stream, 12 MiB/core.
Emission is software-pipelined one chunk deep (DVE diags and the sm of
GPS-carrying chunks are emitted after the next chunk's scores) so no
engine waits at a chunk barrier.
"""

import sys

for _p in ("/opt/trn_rl_repo", "/opt/pypackages"):
    if _p not in sys.path:
        sys.path.append(_p)

import numpy as np

B, S, P, D_SENT, D_CTX = 4, 512, 32, 1024, 768
N_CORES = 8
TOK = B * S                    # 2048
TPC = TOK // N_CORES           # 256 tokens per core
BLK = 128                      # tokens per block
NBLK = TPC // BLK              # 2
CH = 8                         # protos per chunk
NCH = P // CH                  # 4 chunks per block
EH = D_CTX // 2                # 384 = PSUM-bank-sized half
DS = D_SENT // 128             # 8 contraction chunks for the projection

# tuning knobs: per chunk index, how many protos go to GPSIMD (from the
# front of the chunk) and which chunk offsets build their diag on ACT.
GPS_N = {0: 0, 1: 5, 2: 4, 3: 4}
DIAG_ACT = {0: (0, 1, 3, 5, 7), 1: (1, 3, 5, 7), 2: (1, 3, 5, 7),
            3: (0, 1, 2, 3, 4, 5, 6, 7)}

_NC = None


def _build():
    import concourse.tile as tile
    from concourse import bacc, mybir

    f32 = mybir.dt.float32
    f16 = mybir.dt.float16
    bf16 = mybir.dt.bfloat16
    f8 = mybir.dt.float8e4
    Alu = mybir.AluOpType
    Act = mybir.ActivationFunctionType
    X = mybir.AxisListType.X

    nc = bacc.Bacc("TRN2", target_bir_lowering=False)

    sentT_d = nc.dram_tensor("sentT", [D_SENT, TPC], f16, kind="ExternalInput")
    proto_d = nc.dram_tensor("proto", [TPC, P, D_CTX], f16, kind="ExternalInput")
    w_d = nc.dram_tensor("w", [D_SENT, D_CTX], f16, kind="ExternalInput")
    bp_d = nc.dram_tensor("bp", [1, D_CTX], f16, kind="ExternalInput")
    eye_d = nc.dram_tensor("eye", [128, 128], bf16, kind="ExternalInput")
    out_d = nc.dram_tensor("out", [TPC, D_CTX], f16, kind="ExternalOutput")

    with tile.TileContext(nc) as tc:
        with (
            tc.tile_pool(name="persist", bufs=1) as persist,
            tc.tile_pool(name="wpool", bufs=1) as wpool,
            tc.tile_pool(name="ppool", bufs=8) as ppool,
            tc.tile_pool(name="dpool_a", bufs=8) as dpool_a,
            tc.tile_pool(name="dpool_v", bufs=6) as dpool_v,
            tc.tile_pool(name="junk_v", bufs=2) as junk_v,
            tc.tile_pool(name="junk_a", bufs=2) as junk_a,
            tc.tile_pool(name="gsp", bufs=3) as gsp,
            tc.tile_pool(name="small", bufs=6) as small,
            tc.tile_pool(name="psum", bufs=8, space="PSUM") as psum,
        ):
            scores = persist.tile([128, NBLK, P], f32)
            sm = persist.tile([128, NBLK, P], f32)       # clamped, shifted
            expw = persist.tile([128, NBLK, P], f32)
            negM = persist.tile([128, NBLK, 1], f32)
            clampv = persist.tile([128, NBLK, 1], f32)
            qp_sb = persist.tile([128, NBLK, D_CTX], f16)
            out_sb = persist.tile([128, NBLK, D_CTX], f16)

            # ---------------- weights + projection --------------------
            sentT_sb = wpool.tile([128, DS, TPC], f16)
            nc.sync.dma_start(
                out=sentT_sb[:],
                in_=sentT_d[:].rearrange("(dd p) n -> p dd n", p=128),
            )
            w_sb = wpool.tile([128, DS, D_CTX], f16)
            nc.sync.dma_start(
                out=w_sb[:, 0:DS // 2],
                in_=w_d[:].rearrange("(dd p) e -> p dd e", p=128)[:, 0:DS // 2],
            )
            nc.sync.dma_start(
                out=w_sb[:, DS // 2:],
                in_=w_d[:].rearrange("(dd p) e -> p dd e", p=128)[:, DS // 2:],
            )
            bp_sb = wpool.tile([1, D_CTX], f16)
            nc.sync.dma_start(out=bp_sb[:], in_=bp_d[:])
            eye_sb = wpool.tile([128, 128], bf16)
            nc.sync.dma_start(out=eye_sb[:], in_=eye_d[:])
            eyeneg_sb = wpool.tile([128, 128], f32)
            nc.vector.tensor_scalar(
                eyeneg_sb[:], eye_sb[:], 60000.0, -60000.0,
                Alu.mult, Alu.add,
            )
            ones_sb = wpool.tile([1, 128], f16)
            nc.vector.memset(ones_sb[:], 1.0)

            for b in range(NBLK):
                for h in range(2):
                    pp = psum.tile([128, EH], f32, tag="ps")
                    for dd in range(DS):
                        nc.tensor.matmul(
                            pp[:],
                            sentT_sb[:, dd, b * BLK:(b + 1) * BLK],
                            w_sb[:, dd, h * EH:(h + 1) * EH],
                            start=(dd == 0),
                            stop=False,
                        )
                    nc.tensor.matmul(
                        pp[:],
                        ones_sb[0:1, :],
                        bp_sb[0:1, h * EH:(h + 1) * EH],
                        start=False,
                        stop=True,
                    )
                    nc.scalar.copy(
                        out=qp_sb[:, b, h * EH:(h + 1) * EH], in_=pp[:]
                    )

            # ---------------- online softmax-pooling ------------------
            ks = [(b, c) for b in range(NBLK) for c in range(NCH)]
            tiles = {}
            Upsum = {}

            gs_tiles = {}

            def emit_products(k):
                """DMA the chunk tile; GPSIMD products; DVE stt scores.
                ACT accums for the GPSIMD protos are emitted separately
                (emit_accums) so ready diag work can precede them in the
                ACT program."""
                b, c = ks[k]
                p0 = c * CH
                T = ppool.tile([128, CH, D_CTX], f16, tag="T")
                dma_eng = nc.scalar if k < 2 else nc.sync
                dma_eng.dma_start(
                    out=T[:],
                    in_=proto_d[b * BLK:(b + 1) * BLK, p0:p0 + CH, :],
                )
                tiles[k] = T
                ng = GPS_N[c]
                for j in range(ng):
                    gs = gsp.tile([128, D_CTX], f16, tag="gs")
                    nc.gpsimd.tensor_tensor(
                        out=gs[:], in0=T[:, j, :], in1=qp_sb[:, b, :],
                        op=Alu.mult,
                    )
                    gs_tiles[(k, j)] = gs
                for j in range(ng, CH):
                    p = p0 + j
                    jk = junk_v.tile([128, D_CTX], f8, tag="jv")
                    nc.vector.scalar_tensor_tensor(
                        out=jk[:],
                        in0=T[:, j, :],
                        scalar=0.0,
                        in1=qp_sb[:, b, :],
                        op0=Alu.bypass,
                        op1=Alu.mult,
                        accum_out=scores[:, b, p:p + 1],
                    )
                if c == 0:
                    m8 = small.tile([128, 1], f32, tag="m8")
                    nc.vector.tensor_reduce(
                        out=m8[:], in_=scores[:, b, 0:CH], axis=X, op=Alu.max,
                    )
                    nc.vector.tensor_scalar(
                        negM[:, b, :], m8[:], -1.0, -60.0, Alu.mult, Alu.add,
                    )
                    nc.vector.tensor_scalar(
                        clampv[:, b, :], m8[:], 1.0, 140.0, Alu.mult, Alu.add,
                    )

            def emit_accums(k):
                b, c = ks[k]
                p0 = c * CH
                for j in range(GPS_N[c]):
                    p = p0 + j
                    jk = junk_a.tile([128, D_CTX], f8, tag="ja")
                    nc.scalar.activation(
                        out=jk[:], in_=gs_tiles.pop((k, j)), func=Act.Copy,
                        accum_out=scores[:, b, p:p + 1],
                    )

            def emit_sm(k):
                # sm = min(s, clamp) + negM, then expw for Z + ACT diags
                b, c = ks[k]
                p0 = c * CH
                nc.vector.tensor_scalar(
                    sm[:, b, p0:p0 + CH], scores[:, b, p0:p0 + CH],
                    clampv[:, b, :], negM[:, b, :], Alu.min, Alu.add,
                )
                nc.scalar.activation(
                    out=expw[:, b, p0:p0 + CH], in_=sm[:, b, p0:p0 + CH],
                    func=Act.Exp, bias=0.0, scale=1.0,
                )

            dgs = {}

            def emit_diag_act(k):
                b, c = ks[k]
                p0 = c * CH
                for j in DIAG_ACT[c]:
                    dg = dpool_a.tile([128, 128], bf16, tag="dga")
                    nc.scalar.activation(
                        out=dg[:], in_=eyeneg_sb[:], func=Act.Exp,
                        bias=sm[:, b, p0 + j:p0 + j + 1], scale=1.0,
                    )
                    dgs[(k, j)] = dg

            def emit_diag_dve(k):
                b, c = ks[k]
                p0 = c * CH
                for j in range(CH):
                    if j in DIAG_ACT[c]:
                        continue
                    dg = dpool_v.tile([128, 128], bf16, tag="dgv")
                    nc.vector.tensor_scalar(
                        dg[:], eye_sb[:], expw[:, b, p0 + j:p0 + j + 1],
                        None, Alu.mult,
                    )
                    dgs[(k, j)] = dg

            def emit_mac(k):
                b, c = ks[k]
                if c == 0:
                    ulo = psum.tile([128, EH], f32, tag="ps")
                    uhi = psum.tile([128, EH], f32, tag="ps")
                    Upsum[b] = (ulo, uhi)
                ulo, uhi = Upsum[b]
                T = tiles[k]
                order = list(DIAG_ACT[c]) + [
                    j for j in range(CH) if j not in DIAG_ACT[c]
                ]
                for i, j in enumerate(order):
                    dg = dgs.pop((k, j))
                    first = (c == 0 and i == 0)
                    last = (c == NCH - 1 and i == CH - 1)
                    nc.tensor.matmul(
                        ulo[:], dg[:], T[:, j, 0:EH],
                        start=first, stop=last,
                    )
                    nc.tensor.matmul(
                        uhi[:], dg[:], T[:, j, EH:],
                        start=first, stop=last,
                    )

            def emit_final(b):
                z = small.tile([128, 1], f32, tag="z")
                nc.vector.tensor_reduce(
                    out=z[:], in_=expw[:, b, :], axis=X, op=Alu.add,
                )
                rinv = small.tile([128, 1], f32, tag="rinv")
                nc.vector.reciprocal(out=rinv[:], in_=z[:])
                ulo, uhi = Upsum[b]
                nc.scalar.activation(
                    out=out_sb[:, b, 0:EH], in_=ulo[:], func=Act.Copy,
                    scale=rinv[:],
                )
                nc.scalar.activation(
                    out=out_sb[:, b, EH:], in_=uhi[:], func=Act.Copy,
                    scale=rinv[:],
                )
                nc.sync.dma_start(
                    out=out_d[b * BLK:(b + 1) * BLK, :], in_=out_sb[:, b, :]
                )

            # one-chunk-deep software pipeline; per iteration k:
            #   1. products+stt scores for chunk k (GPS + DVE)
            #   2. sm(k-1) on DVE (its ACT accums are well underway)
            #   3. ACT diags + exp for chunk k-1 (ready work first in the
            #      ACT program), then the ACT accums for chunk k
            #   4. DVE diags for k-1, then MAC(k-1) on the PE
            # c==0 chunks have no GPSIMD protos so their sm/ACT-diags
            # happen immediately (the frame comes from their scores).
            def post_scores(k):
                emit_sm(k)
                emit_diag_act(k)

            for k in range(len(ks)):
                b, c = ks[k]
                emit_products(k)
                prev = k - 1
                if prev >= 0:
                    if GPS_N[ks[prev][1]] != 0:
                        post_scores(prev)
                    if GPS_N[c] != 0:
                        emit_accums(k)
                    emit_diag_dve(prev)
                    emit_mac(prev)
                    if ks[prev][1] == NCH - 1:
                        emit_final(ks[prev][0])
                if c == 0:
                    post_scores(k)
            last = len(ks) - 1
            if GPS_N[ks[last][1]] != 0:
                post_scores(last)
            emit_diag_dve(last)
            emit_mac(last)
            emit_final(ks[last][0])

    nc.compile()
    return nc


def _get_nc():
    global _NC
    if _NC is None:
        _NC = _build()
    return _NC


def _make_in_maps(sent_vecs, proto_vecs, Wq, bq, Wk):
    f16 = np.float16
    import ml_dtypes

    sent = np.asarray(sent_vecs, dtype=np.float32).reshape(TOK, D_SENT)
    sentT = np.ascontiguousarray(sent.T.astype(f16))          # [D_SENT, TOK]
    proto = np.asarray(proto_vecs, dtype=np.float32).reshape(TOK, P, D_CTX)
    proto16 = np.ascontiguousarray(proto.astype(f16))
    wq = np.asarray(Wq, dtype=np.float32)
    bq = np.asarray(bq, dtype=np.float32).reshape(1, D_CTX)
    wk = np.asarray(Wk, dtype=np.float32)
    w = np.ascontiguousarray((wq @ wk.T).astype(f16))
    bp = np.ascontiguousarray((bq @ wk.T).astype(f16))
    eye = np.ascontiguousarray(np.eye(128, dtype=ml_dtypes.bfloat16))
    in_maps = []
    for i in range(N_CORES):
        sl = slice(i * TPC, (i + 1) * TPC)
        in_maps.append(
            {
                "sentT": np.ascontiguousarray(sentT[:, sl]),
                "proto": np.ascontiguousarray(proto16[sl]),
                "w": w,
                "bp": bp,
                "eye": eye,
            }
        )
    return in_maps


def _ensure_ntff_hook():
    """The agent image's antenv lacks axon_hooks; shim it so trace=True
    can capture NTFF profiles via the libaxon ctypes path."""
    try:
        from antenv.axon_hooks import get_axon_ntff_profile_hook  # noqa: F401
        return
    except ImportError:
        pass
    import types

    import antenv
    from trn_agent_boot.trn_boot import _ntff_profile_via_ctypes

    mod = types.ModuleType("antenv.axon_hooks")
    mod._hook = _ntff_profile_via_ctypes("/opt/axon/libaxon_pjrt.so")
    mod.get_axon_ntff_profile_hook = lambda: mod._hook
    mod.set_axon_ntff_profile_hook = lambda h: setattr(mod, "_hook", h)
    sys.modules["antenv.axon_hooks"] = mod
    antenv.axon_hooks = mod


def run(sent_vecs, proto_vecs, Wq, bq, Wk, bk=None, trace=False, **kw):
    """Returns (out[4,512,768] float32, BassKernelResults)."""
    from concourse.bass_utils import run_bass_kernel_spmd

    if trace:
        _ensure_ntff_hook()
    nc = _get_nc()
    in_maps = _make_in_maps(sent_vecs, proto_vecs, Wq, bq, Wk)
    res = run_bass_kernel_spmd(
        nc, in_maps, core_ids=list(range(N_CORES)), trace=trace
    )
    outs = [np.asarray(res.results[i]["out"]) for i in range(N_CORES)]
    full = np.concatenate(outs, axis=0).reshape(B, S, D_CTX).astype(np.float32)
    return full, res


def kernel(sent_vecs, proto_vecs, Wq, bq, Wk, bk=None, **kw):
    out, _ = run(sent_vecs, proto_vecs, Wq, bq, Wk, bk)
    return out


if __name__ == "__main__":
    nc = _get_nc()
    print("build + compile OK")


# revision 10
# speedup vs baseline: 1.2729x; 1.2729x over previous
"""AttentiveProtoFusion kernel for 8 TRN2 NeuronCores.

Math (equivalent to reference, ~14x fewer FLOPs):
    q' = sent @ (Wq @ Wk^T) + bq @ Wk^T      [n, 768]   (folded host-side)
    scores[n,p] = sum_c proto[n,p,c] * q'[n,c]
    w = softmax(scores, axis=p);  ctx[n,c] = sum_p w[n,p] * proto[n,p,c]

Sharding: data-parallel over the 2048 tokens (B*S), 256/core, 2 blocks of
128 tokens (tokens on partitions). proto/sent/W staged host-side in fp16
(rel err ~2.8e-3 vs the 2e-2 gate; halves DMA to 12 MiB/core).

Softmax uses the fixed exponent frame proven in the fp32 baseline:
Mhat = max(chunk0) + 60, scores clamped at Mhat + 80, so e = exp(s - Mhat)
spans up to e^80 = 5.5e34 - safely inside bf16 range. The pooling weights
e are materialised as bf16 DIAGONAL matrices and the whole MAC
U += e_p * proto_p runs on the TensorEngine as
matmul(lhsT=diag(e_p) bf16, rhs=proto_p fp16) accumulating in PSUM fp32
(mixed 16-bit dtypes verified exact on HW; ldweights pipelines behind the
previous matmul, 163 ns/384-col matmul at full clock).

Engine plan (measured costs):
  DVE   : score passes via fused scalar_tensor_tensor+accum (~1.1 us);
          some diag builds diag=TS(eye*e_p) (~345 ns); softmax sm/frames.
  GPSIMD: score products for ~13 of 32 protos per block (TT ~1.65 us).
  ACT   : accumulates GPSIMD products (~1.3 us); per-chunk exp for Z;
          other diag builds via dg=Exp(eyeNEG + sm_p) bf16 (~385 ns,
          eyeNEG has -60000 off-diagonal so exp -> 0); q' copies; final
          ctx = U * (1/Z) PSUM->SBUF copies.
  PE    : q' projection + the 128 MAC matmuls/block.
  DMA   : proto fp16 stream, 12 MiB/core.
Emission is software-pipelined one chunk deep (DVE diags and the sm of
GPS-carrying chunks are emitted after the next chunk's scores) so no
engine waits at a chunk barrier.
"""

import sys

for _p in ("/opt/trn_rl_repo", "/opt/pypackages"):
    if _p not in sys.path:
        sys.path.append(_p)

import numpy as np

B, S, P, D_SENT, D_CTX = 4, 512, 32, 1024, 768
N_CORES = 8
TOK = B * S                    # 2048
TPC = TOK // N_CORES           # 256 tokens per core
BLK = 128                      # tokens per block
NBLK = TPC // BLK              # 2
CH = 8                         # protos per chunk
NCH = P // CH                  # 4 chunks per block
EH = D_CTX // 2                # 384 = PSUM-bank-sized half
DS = D_SENT // 128             # 8 contraction chunks for the projection

# tuning knobs: per chunk index, how many protos go to GPSIMD (from the
# front of the chunk) and which chunk offsets build their diag on ACT.
GPS_N = {0: 0, 1: 5, 2: 4, 3: 4}
DIAG_ACT = {0: (1, 3, 5), 1: (1, 3, 5), 2: (1, 3, 5),
            3: (0, 1, 2, 3, 4, 5, 6, 7)}

_NC = None


def _build():
    import concourse.tile as tile
    from concourse import bacc, mybir

    f32 = mybir.dt.float32
    f16 = mybir.dt.float16
    bf16 = mybir.dt.bfloat16
    f8 = mybir.dt.float8e4
    Alu = mybir.AluOpType
    Act = mybir.ActivationFunctionType
    X = mybir.AxisListType.X

    nc = bacc.Bacc("TRN2", target_bir_lowering=False)

    sentT_d = nc.dram_tensor("sentT", [D_SENT, TPC], f16, kind="ExternalInput")
    proto_d = nc.dram_tensor("proto", [TPC, P, D_CTX], f16, kind="ExternalInput")
    w_d = nc.dram_tensor("w", [D_SENT, D_CTX], f16, kind="ExternalInput")
    bp_d = nc.dram_tensor("bp", [1, D_CTX], f16, kind="ExternalInput")
    eye_d = nc.dram_tensor("eye", [128, 128], bf16, kind="ExternalInput")
    out_d = nc.dram_tensor("out", [TPC, D_CTX], f16, kind="ExternalOutput")

    with tile.TileContext(nc) as tc:
        with (
            tc.tile_pool(name="persist", bufs=1) as persist,
            tc.tile_pool(name="wpool", bufs=1) as wpool,
            tc.tile_pool(name="ppool", bufs=8) as ppool,
            tc.tile_pool(name="dpool_a", bufs=8) as dpool_a,
            tc.tile_pool(name="dpool_v", bufs=6) as dpool_v,
            tc.tile_pool(name="junk_v", bufs=2) as junk_v,
            tc.tile_pool(name="junk_a", bufs=2) as junk_a,
            tc.tile_pool(name="gsp", bufs=3) as gsp,
            tc.tile_pool(name="small", bufs=6) as small,
            tc.tile_pool(name="psum", bufs=8, space="PSUM") as psum,
        ):
            scores = persist.tile([128, NBLK, P], f32)
            sm = persist.tile([128, NBLK, P], f32)       # clamped, shifted
            expw = persist.tile([128, NBLK, P], f32)
            negM = persist.tile([128, NBLK, 1], f32)
            clampv = persist.tile([128, NBLK, 1], f32)
            qp_sb = persist.tile([128, NBLK, D_CTX], f16)
            out_sb = persist.tile([128, NBLK, D_CTX], f16)

            # ---------------- weights + projection --------------------
            sentT_sb = wpool.tile([128, DS, TPC], f16)
            nc.sync.dma_start(
                out=sentT_sb[:],
                in_=sentT_d[:].rearrange("(dd p) n -> p dd n", p=128),
            )
            w_sb = wpool.tile([128, DS, D_CTX], f16)
            nc.sync.dma_start(
                out=w_sb[:, 0:DS // 2],
                in_=w_d[:].rearrange("(dd p) e -> p dd e", p=128)[:, 0:DS // 2],
            )
            nc.sync.dma_start(
                out=w_sb[:, DS // 2:],
                in_=w_d[:].rearrange("(dd p) e -> p dd e", p=128)[:, DS // 2:],
            )
            bp_sb = wpool.tile([1, D_CTX], f16)
            nc.sync.dma_start(out=bp_sb[:], in_=bp_d[:])
            eye_sb = wpool.tile([128, 128], bf16)
            nc.sync.dma_start(out=eye_sb[:], in_=eye_d[:])
            eyeneg_sb = wpool.tile([128, 128], f32)
            nc.vector.tensor_scalar(
                eyeneg_sb[:], eye_sb[:], 60000.0, -60000.0,
                Alu.mult, Alu.add,
            )
            ones_sb = wpool.tile([1, 128], f16)
            nc.vector.memset(ones_sb[:], 1.0)

            for b in range(NBLK):
                for h in range(2):
                    pp = psum.tile([128, EH], f32, tag="ps")
                    for dd in range(DS):
                        nc.tensor.matmul(
                            pp[:],
                            sentT_sb[:, dd, b * BLK:(b + 1) * BLK],
                            w_sb[:, dd, h * EH:(h + 1) * EH],
                            start=(dd == 0),
                            stop=False,
                        )
                    nc.tensor.matmul(
                        pp[:],
                        ones_sb[0:1, :],
                        bp_sb[0:1, h * EH:(h + 1) * EH],
                        start=False,
                        stop=True,
                    )
                    nc.scalar.copy(
                        out=qp_sb[:, b, h * EH:(h + 1) * EH], in_=pp[:]
                    )

            # ---------------- online softmax-pooling ------------------
            ks = [(b, c) for b in range(NBLK) for c in range(NCH)]
            tiles = {}
            Upsum = {}

            gs_tiles = {}

            def emit_products(k):
                """DMA the chunk tile; GPSIMD products; DVE stt scores.
                ACT accums for the GPSIMD protos are emitted separately
                (emit_accums) so ready diag work can precede them in the
                ACT program."""
                b, c = ks[k]
                p0 = c * CH
                T = ppool.tile([128, CH, D_CTX], f16, tag="T")
                nc.sync.dma_start(
                    out=T[:],
                    in_=proto_d[b * BLK:(b + 1) * BLK, p0:p0 + CH, :],
                )
                tiles[k] = T
                ng = GPS_N[c]
                for j in range(ng):
                    gs = gsp.tile([128, D_CTX], f16, tag="gs")
                    nc.gpsimd.tensor_tensor(
                        out=gs[:], in0=T[:, j, :], in1=qp_sb[:, b, :],
                        op=Alu.mult,
                    )
                    gs_tiles[(k, j)] = gs
                for j in range(ng, CH):
                    p = p0 + j
                    jk = junk_v.tile([128, D_CTX], f16, tag="jv")
                    nc.vector.scalar_tensor_tensor(
                        out=jk[:],
                        in0=T[:, j, :],
                        scalar=0.0,
                        in1=qp_sb[:, b, :],
                        op0=Alu.bypass,
                        op1=Alu.mult,
                        accum_out=scores[:, b, p:p + 1],
                    )
                if c == 0:
                    m8 = small.tile([128, 1], f32, tag="m8")
                    nc.vector.tensor_reduce(
                        out=m8[:], in_=scores[:, b, 0:CH], axis=X, op=Alu.max,
                    )
                    nc.vector.tensor_scalar(
                        negM[:, b, :], m8[:], -1.0, -60.0, Alu.mult, Alu.add,
                    )
                    nc.vector.tensor_scalar(
                        clampv[:, b, :], m8[:], 1.0, 140.0, Alu.mult, Alu.add,
                    )

            def emit_accums(k):
                b, c = ks[k]
                p0 = c * CH
                for j in range(GPS_N[c]):
                    p = p0 + j
                    jk = junk_a.tile([128, D_CTX], f16, tag="ja")
                    nc.scalar.activation(
                        out=jk[:], in_=gs_tiles.pop((k, j)), func=Act.Copy,
                        accum_out=scores[:, b, p:p + 1],
                    )

            def emit_sm(k):
                # sm = min(s, clamp) + negM, then expw for Z + ACT diags
                b, c = ks[k]
                p0 = c * CH
                nc.vector.tensor_scalar(
                    sm[:, b, p0:p0 + CH], scores[:, b, p0:p0 + CH],
                    clampv[:, b, :], negM[:, b, :], Alu.min, Alu.add,
                )
                nc.scalar.activation(
                    out=expw[:, b, p0:p0 + CH], in_=sm[:, b, p0:p0 + CH],
                    func=Act.Exp, bias=0.0, scale=1.0,
                )

            dgs = {}

            def emit_diag_act(k):
                b, c = ks[k]
                p0 = c * CH
                for j in DIAG_ACT[c]:
                    dg = dpool_a.tile([128, 128], bf16, tag="dga")
                    nc.scalar.activation(
                        out=dg[:], in_=eyeneg_sb[:], func=Act.Exp,
                        bias=sm[:, b, p0 + j:p0 + j + 1], scale=1.0,
                    )
                    dgs[(k, j)] = dg

            def emit_diag_dve(k):
                b, c = ks[k]
                p0 = c * CH
                for j in range(CH):
                    if j in DIAG_ACT[c]:
                        continue
                    dg = dpool_v.tile([128, 128], bf16, tag="dgv")
                    nc.vector.tensor_scalar(
                        dg[:], eye_sb[:], expw[:, b, p0 + j:p0 + j + 1],
                        None, Alu.mult,
                    )
                    dgs[(k, j)] = dg

            def emit_mac(k):
                b, c = ks[k]
                if c == 0:
                    ulo = psum.tile([128, EH], f32, tag="ps")
                    uhi = psum.tile([128, EH], f32, tag="ps")
                    Upsum[b] = (ulo, uhi)
                ulo, uhi = Upsum[b]
                T = tiles[k]
                order = list(DIAG_ACT[c]) + [
                    j for j in range(CH) if j not in DIAG_ACT[c]
                ]
                for i, j in enumerate(order):
                    dg = dgs.pop((k, j))
                    first = (c == 0 and i == 0)
                    last = (c == NCH - 1 and i == CH - 1)
                    nc.tensor.matmul(
                        ulo[:], dg[:], T[:, j, 0:EH],
                        start=first, stop=last,
                    )
                    nc.tensor.matmul(
                        uhi[:], dg[:], T[:, j, EH:],
                        start=first, stop=last,
                    )

            def emit_final(b):
                z = small.tile([128, 1], f32, tag="z")
                nc.vector.tensor_reduce(
                    out=z[:], in_=expw[:, b, :], axis=X, op=Alu.add,
                )
                rinv = small.tile([128, 1], f32, tag="rinv")
                nc.vector.reciprocal(out=rinv[:], in_=z[:])
                ulo, uhi = Upsum[b]
                nc.scalar.activation(
                    out=out_sb[:, b, 0:EH], in_=ulo[:], func=Act.Copy,
                    scale=rinv[:],
                )
                nc.scalar.activation(
                    out=out_sb[:, b, EH:], in_=uhi[:], func=Act.Copy,
                    scale=rinv[:],
                )
                nc.sync.dma_start(
                    out=out_d[b * BLK:(b + 1) * BLK, :], in_=out_sb[:, b, :]
                )

            # one-chunk-deep software pipeline; per iteration k:
            #   1. products+stt scores for chunk k (GPS + DVE)
            #   2. sm(k-1) on DVE (its ACT accums are well underway)
            #   3. ACT diags + exp for chunk k-1 (ready work first in the
            #      ACT program), then the ACT accums for chunk k
            #   4. DVE diags for k-1, then MAC(k-1) on the PE
            # c==0 chunks have no GPSIMD protos so their sm/ACT-diags
            # happen immediately (the frame comes from their scores).
            def post_scores(k):
                emit_sm(k)
                emit_diag_act(k)

            for k in range(len(ks)):
                b, c = ks[k]
                emit_products(k)
                prev = k - 1
                if prev >= 0:
                    if GPS_N[ks[prev][1]] != 0:
                        post_scores(prev)
                    if GPS_N[c] != 0:
                        emit_accums(k)
                    emit_diag_dve(prev)
                    emit_mac(prev)
                    if ks[prev][1] == NCH - 1:
                        emit_final(ks[prev][0])
                if c == 0:
                    post_scores(k)
            last = len(ks) - 1
            if GPS_N[ks[last][1]] != 0:
                post_scores(last)
            emit_diag_dve(last)
            emit_mac(last)
            emit_final(ks[last][0])

    nc.compile()
    return nc


def _get_nc():
    global _NC
    if _NC is None:
        _NC = _build()
    return _NC


def _make_in_maps(sent_vecs, proto_vecs, Wq, bq, Wk):
    f16 = np.float16
    import ml_dtypes

    sent = np.asarray(sent_vecs, dtype=np.float32).reshape(TOK, D_SENT)
    sentT = np.ascontiguousarray(sent.T.astype(f16))          # [D_SENT, TOK]
    proto = np.asarray(proto_vecs, dtype=np.float32).reshape(TOK, P, D_CTX)
    proto16 = np.ascontiguousarray(proto.astype(f16))
    wq = np.asarray(Wq, dtype=np.float32)
    bq = np.asarray(bq, dtype=np.float32).reshape(1, D_CTX)
    wk = np.asarray(Wk, dtype=np.float32)
    w = np.ascontiguousarray((wq @ wk.T).astype(f16))
    bp = np.ascontiguousarray((bq @ wk.T).astype(f16))
    eye = np.ascontiguousarray(np.eye(128, dtype=ml_dtypes.bfloat16))
    in_maps = []
    for i in range(N_CORES):
        sl = slice(i * TPC, (i + 1) * TPC)
        in_maps.append(
            {
                "sentT": np.ascontiguousarray(sentT[:, sl]),
                "proto": np.ascontiguousarray(proto16[sl]),
                "w": w,
                "bp": bp,
                "eye": eye,
            }
        )
    return in_maps


def _ensure_ntff_hook():
    """The agent image's antenv lacks axon_hooks; shim it so trace=True
    can capture NTFF profiles via the libaxon ctypes path."""
    try:
        from antenv.axon_hooks import get_axon_ntff_profile_hook  # noqa: F401
        return
    except ImportError:
        pass
    import types

    import antenv
    from trn_agent_boot.trn_boot import _ntff_profile_via_ctypes

    mod = types.ModuleType("antenv.axon_hooks")
    mod._hook = _ntff_profile_via_ctypes("/opt/axon/libaxon_pjrt.so")
    mod.get_axon_ntff_profile_hook = lambda: mod._hook
    mod.set_axon_ntff_profile_hook = lambda h: setattr(mod, "_hook", h)
    sys.modules["antenv.axon_hooks"] = mod
    antenv.axon_hooks = mod


def run(sent_vecs, proto_vecs, Wq, bq, Wk, bk=None, trace=False, **kw):
    """Returns (out[4,512,768] float32, BassKernelResults)."""
    from concourse.bass_utils import run_bass_kernel_spmd

    if trace:
        _ensure_ntff_hook()
    nc = _get_nc()
    in_maps = _make_in_maps(sent_vecs, proto_vecs, Wq, bq, Wk)
    res = run_bass_kernel_spmd(
        nc, in_maps, core_ids=list(range(N_CORES)), trace=trace
    )
    outs = [np.asarray(res.results[i]["out"]) for i in range(N_CORES)]
    full = np.concatenate(outs, axis=0).reshape(B, S, D_CTX).astype(np.float32)
    return full, res


def kernel(sent_vecs, proto_vecs, Wq, bq, Wk, bk=None, **kw):
    out, _ = run(sent_vecs, proto_vecs, Wq, bq, Wk, bk)
    return out


if __name__ == "__main__":
    nc = _get_nc()
    print("build + compile OK")
